# revision 2
# baseline (speedup 1.0000x reference)
"""AWQLinear forward on 8 Trainium2 NeuronCores.

y = x @ dequant(W)^T + bias, where
  dequant(W)[o,k] = (wint[o,k] - zero[o,g(k)]) * scale[o,g(k)] / awq[k],
  g(k) = k // 128.

Sharding: tensor-parallel over out_features (11008 -> 8 x 1376). x is
replicated; each core dequantizes its weight slice on-device, runs the
matmul in bf16 with fp32 PSUM accumulation, and writes its output
column block. The host concatenates the column blocks.

Layouts fed to the device (all transposes done host-side, so the
contraction dim K=in_features lands on SBUF partitions):
  xT     (4096, 8192) bf16   x transposed (replicated)
  wT     (4096, 1376) bf16   weight_int slice transposed (int4 values, exact in bf16)
  scaleT (32, 1376)   f32    scale_per_group slice transposed
  zeroT  (32, 1376)   f32    zero_per_group slice transposed
  awq    (4096,)      f32
  bias   (1376,)      f32
"""

import os
import sys

for _p in ("/opt/trn_rl_repo", "/opt/pypackages"):
    if os.path.isdir(_p) and _p not in sys.path:
        sys.path.append(_p)

import numpy as np
import ml_dtypes

import concourse.bass as bass
import concourse.mybir as mybir
import concourse.tile as tile
from concourse import bacc
from concourse.bass_utils import run_bass_kernel_spmd

BF16 = ml_dtypes.bfloat16

T_FULL = 8192      # tokens
IN = 4096          # in_features (contraction)
OUT = 11008        # out_features
N_CORES = 8
OUT_S = OUT // N_CORES   # 1376 per core
GS = 128           # quant group size == SBUF partition count
G = IN // GS       # 32 groups

f32 = mybir.dt.float32
bf16 = mybir.dt.bfloat16


def build_nc(t_tokens=T_FULL, out_s=OUT_S, t_chunk=512, mm_dt=bf16):
    """Build the per-core Bass program (same program on all 8 cores)."""
    assert t_tokens % t_chunk == 0 and t_chunk % 128 == 0
    nc = bacc.Bacc("TRN2", target_bir_lowering=False, debug=False)

    xT = nc.dram_tensor("xT", [IN, t_tokens], mm_dt, kind="ExternalInput").ap()
    wT = nc.dram_tensor("wT", [IN, out_s], bf16, kind="ExternalInput").ap()
    scaleT = nc.dram_tensor("scaleT", [G, out_s], f32, kind="ExternalInput").ap()
    zeroT = nc.dram_tensor("zeroT", [G, out_s], f32, kind="ExternalInput").ap()
    awq = nc.dram_tensor("awq", [IN], f32, kind="ExternalInput").ap()
    bias = nc.dram_tensor("bias", [out_s], f32, kind="ExternalInput").ap()
    y = nc.dram_tensor("y", [t_tokens, out_s], f32, kind="ExternalOutput").ap()

    # output column chunks, each <= 512 (one PSUM bank)
    o_chunks = []
    o0 = 0
    while o0 < out_s:
        o_chunks.append((o0, min(o0 + 512, out_s)))
        o0 += 512

    with tile.TileContext(nc) as tc:
        with tc.tile_pool(name="consts", bufs=1) as consts:
            awq_sb = consts.tile([GS, G], f32)
            nc.sync.dma_start(awq_sb[:], awq.rearrange("(g p) -> p g", p=GS))
            awq_inv = consts.tile([GS, G], f32)
            nc.vector.reciprocal(awq_inv[:], awq_sb[:])
            bias_b = consts.tile([GS, out_s], f32)
            nc.sync.dma_start(bias_b[:], bias[None, :].to_broadcast([GS, out_s]))
            # resident dequantized weights, [k_in_group, group, out]
            wp = consts.tile([GS, G, out_s], mm_dt)

            # --- dequantize weights, one 128-row group at a time ---
            with tc.tile_pool(name="dq", bufs=2) as dq:
                for g in range(G):
                    wint_t = dq.tile([GS, out_s], bf16, tag="wint")
                    nc.sync.dma_start(wint_t[:], wT[g * GS:(g + 1) * GS, :])
                    sc_b = dq.tile([GS, out_s], f32, tag="sc")
                    nc.sync.dma_start(
                        sc_b[:], scaleT[g:g + 1, :].to_broadcast([GS, out_s]))
                    z_b = dq.tile([GS, out_s], f32, tag="z")
                    nc.sync.dma_start(
                        z_b[:], zeroT[g:g + 1, :].to_broadcast([GS, out_s]))
                    t0 = dq.tile([GS, out_s], f32, tag="t0")
                    nc.vector.tensor_sub(t0[:], wint_t[:], z_b[:])
                    # wp = (t0 * awq_inv[k]) * scale_bcast
                    nc.vector.scalar_tensor_tensor(
                        wp[:, g, :], t0[:], awq_inv[:, g:g + 1], sc_b[:],
                        mybir.AluOpType.mult, mybir.AluOpType.mult)

            # --- matmul: out[t,o] = sum_k xT[k,t] * wp[k,o] ---
            xT3 = xT.rearrange("(g p) t -> p g t", p=GS)
            n_tt = t_chunk // 128
            with (
                tc.tile_pool(name="xp", bufs=2) as xp,
                tc.tile_pool(name="outp", bufs=3) as outp,
                tc.tile_pool(name="ps", bufs=2 * len(o_chunks), space="PSUM") as ps,
            ):
                for c in range(t_tokens // t_chunk):
                    x_sb = xp.tile([GS, G, t_chunk], mm_dt, tag="x")
                    nc.sync.dma_start(
                        x_sb[:], xT3[:, :, c * t_chunk:(c + 1) * t_chunk])
                    for tt in range(n_tt):
                        trow = c * t_chunk + tt * 128
                        psums = [ps.tile([128, 512], f32, tag="ps",
                                         name=f"ps_{c}_{tt}_{i}")
                                 for i in range(len(o_chunks))]
                        for k in range(G):
                            lhsT = x_sb[:, k, tt * 128:(tt + 1) * 128]
                            for oc, (a, b) in enumerate(o_chunks):
                                nc.tensor.matmul(
                                    psums[oc][:, :b - a], lhsT, wp[:, k, a:b],
                                    start=(k == 0), stop=(k == G - 1))
                        out_sb = outp.tile([128, out_s], f32, tag="out")
                        for oc, (a, b) in enumerate(o_chunks):
                            nc.any.tensor_add(
                                out_sb[:, a:b], psums[oc][:, :b - a],
                                bias_b[:, a:b])
                        nc.sync.dma_start(y[trow:trow + 128, :], out_sb[:])

    nc.compile()
    return nc


def make_in_maps(x, weight_int, scale_per_group, zero_per_group, awq_scale,
                 bias, out_s=OUT_S, n_cores=N_CORES):
    """Shard + lay out host inputs for the 8 cores."""
    xT = np.ascontiguousarray(np.asarray(x, dtype=np.float32).astype(BF16).T)
    awq_f = np.ascontiguousarray(np.asarray(awq_scale, dtype=np.float32))
    in_maps = []
    for s in range(n_cores):
        sl = slice(s * out_s, (s + 1) * out_s)
        in_maps.append({
            "xT": xT,
            # int4 values are exact in bf16
            "wT": np.ascontiguousarray(
                np.asarray(weight_int)[sl].T.astype(BF16)),
            "scaleT": np.ascontiguousarray(
                np.asarray(scale_per_group, dtype=np.float32)[sl].T),
            "zeroT": np.ascontiguousarray(
                np.asarray(zero_per_group, dtype=np.float32)[sl].T),
            "awq": awq_f,
            "bias": np.ascontiguousarray(
                np.asarray(bias, dtype=np.float32)[sl]),
        })
    return in_maps


_NC_CACHE = {}


def _get_nc():
    key = (T_FULL, OUT_S)
    if key not in _NC_CACHE:
        _NC_CACHE[key] = build_nc()
    return _NC_CACHE[key]


def kernel(x, weight_int, scale_per_group, zero_per_group, awq_scale, bias,
           **_kw):
    in_maps = make_in_maps(x, weight_int, scale_per_group, zero_per_group,
                           awq_scale, bias)
    nc = _get_nc()
    res = run_bass_kernel_spmd(nc, in_maps, core_ids=list(range(N_CORES)))
    y = np.concatenate([res.results[s]["y"] for s in range(N_CORES)], axis=1)
    return np.ascontiguousarray(y, dtype=np.float32)
